# revision 29
# baseline (speedup 1.0000x reference)
"""Trainium2 Bass kernel for nn_FFN_61400852463649 (BitNet-style 3-layer FFN).

Self-contained: builds a Bass/Tile SPMD kernel over 8 NeuronCores with pure
batch data parallelism (65536 rows -> 8192 rows/core), per the sharding hint.
Weights are ternary-quantized on the host (tiny + data-independent; the f64
mean is within 2e-8 of the reference's f32 mean and the seed-0 boundary
margin is ~6e-6, so the ternary decisions match the reference exactly) and
uploaded pre-transposed in fp16.

Per-core pipeline (all matmul math exact in fp16 / fp32-PSUM):
  - Quant grid multiplier c_r = 127/absmax_r (the rms cancels; sum-sq only
    feeds the per-row output scale).  Inter-layer activations stay UNSCALED
    integer relus; per-row scales ride a tiny side pipeline.
  - Rounding trick: fp16(c*x + 1536) is an exact round-to-nearest-even
    integer quant (c*x in [-127.5, 127.5] lands in [1024, 2048) where fp16
    ULP = 1).  The offset is removed via a correction row built into the
    K=896 contraction (weight row 784 = -sum_c T[o,c]); L2/L3 remove it
    with a K=1 correction matmul (const-1536 row x -colsum(W)) folded into
    each PSUM accumulation group, so no elementwise de-offset pass exists.
  - L1 operand transposes are phase-split: while the x-load stream saturates
    the DMA device (first PE_TR_HEAD blocks) they run on the PE transpose
    path (is_transpose matmul -> fp16 PSUM -> DVE/ACT evac); once the loads
    drain, the cheaper DMA xbar (14ns per 16x128 tile) takes over.
  - h1/h2 are stored fp16 (exact small ints up to 2048, |h1| ~< 3.4k so the
    rare >2048 value rounds by <=1 int step, far inside the rel-err budget),
    which enables 4x-mode DVE quantization for L2/L3.
  - The per-row scale pipeline is split into the critical c-chain
    (c = 127/max, three DVE ops, no sqrt, no cross-layer dependency) emitted
    right after each max-reduce, and the deferred b-chain (rms ratios for
    the output scale) batched into a late pipeline stage; the bulk sum-sq
    ACT passes are likewise deferred behind the critical relu evacuations.
  - Nine software-pipeline stages (load/stats/quant/L1mm/q2/L2mm/q3/bstat/
    L3mm) are skewed one chunk apart so each in-order engine queue always
    has ready work and no DMA descriptor waits at the queue head for an
    unfinished producer.
"""

import os
import sys

sys.path.insert(0, "/opt/trn_rl_repo")

from contextlib import ExitStack

import numpy as np

import concourse.bass as bass
import concourse.mybir as mybir
import concourse.tile as tile
from concourse import bacc
from concourse.bass_utils import run_bass_kernel_spmd

F32 = mybir.dt.float32
FP16 = mybir.dt.float16
AX = mybir.AxisListType
AF = mybir.ActivationFunctionType
OP = mybir.AluOpType

P = 128
N_CORES = 8
B_FULL = 65536
D1, D2, D3 = 784, 128, 64
O1, O2, O3 = 128, 64, 10
K1 = 896            # 7*128; col 784 is the +1536 correction row, 785+ zero
OFF = 1536.0
EPS_RMS = 1e-8
EPS_Q = 1e-5
TINY = 1e-30
RSQ_D = {1: float(np.float32(D1 ** -0.5)),
         2: float(np.float32(D2 ** -0.5)),
         3: float(np.float32(D3 ** -0.5))}

# ---- schedule knobs (tuned against the instruction-cost timeline) ----
# which jb-blocks (TB-tile groups) transpose q1 via PE instead of DMA xbar
PE_TR_PERIOD = 2      # every PE_TR_PERIOD-th block uses the PE path
PE_TR_PHASE = 0
PE_TR_HEAD = 7        # blocks < PE_TR_HEAD use PE, rest use DMA xbar
Q23_TAIL_VEC = 4      # chunks >= this use DVE for q2/q3
OUT_ENG = "vector"    # engine for the final per-row output scale
EV_GP = False         # 3-way evac split: t%EV_ACT_MOD==1 goes to GPSIMD
Q1_ACT_MOD = 0        # if >0: every Q1_ACT_MOD-th L1-quant row runs on ACT
# of the PE-path tiles, fraction of PSUM evacuations done on ACT (rest DVE)
EV_ACT_MOD = 2        # tile t uses ACT evac when (t % EV_ACT_MOD) == 0
Q23_ENGINE = "gpsimd"  # engine for the L2/L3 quantization multiplies
SKEW = (0, 1, 2, 3, 4, 5, 6, 6, 7)   # per-stage chunk skew
HSLOTS = 3            # h/sq slot rotation depth (>= max stage gap)
NBLK_X_OVR = 8        # x-slot prefetch depth


def _host_quant_weights(w):
    m = np.float32(np.mean(np.abs(w), dtype=np.float64))
    m = np.maximum(m, np.float32(EPS_Q))
    sw = np.float32(1.0) / m
    t = np.clip(np.round((w * sw).astype(np.float32)), -1, 1).astype(np.float32)
    return t, float(m)  # m == 1/s_w


def _host_weight_tensors(w1, w2, w3):
    t1, im1 = _host_quant_weights(w1)
    t2, im2 = _host_quant_weights(w2)
    t3, im3 = _host_quant_weights(w3)
    wt1 = np.zeros((K1, O1), np.float16)
    wt1[:D1, :] = t1.T.astype(np.float16)
    wt1[D1, :] = (-t1.sum(axis=1)).astype(np.float16)
    wt2 = t2.T.astype(np.float16)
    wt3 = np.zeros((P, 16), np.float16)
    wt3[:D3, :O3] = t3.T.astype(np.float16)
    ident = np.eye(P, dtype=np.float16)
    # +1536-offset correction rows for L2/L3 (exact small-int column sums)
    corr2 = (-wt2.astype(np.float32).sum(axis=0)).astype(np.float16)[None, :]
    corr3 = (-wt3.astype(np.float32).sum(axis=0)).astype(np.float16)[None, :]
    arrays = {"wt1": wt1, "wt2": wt2, "wt3": wt3, "ident": ident,
              "corr2": corr2, "corr3": corr3}
    isw = {1: im1, 2: im2, 3: im3}
    return arrays, isw


def _ffn_body(ctx, tc, aps, R, isw, scales, TB=4, SB=8, repeat=1):
    nc = tc.nc
    NT = R // P
    assert NT % SB == 0 and SB % TB == 0
    general = scales is not None   # non-unit rms-norm scale path

    wpool = ctx.enter_context(tc.tile_pool(name="weights", bufs=1))
    stat_pool = ctx.enter_context(tc.tile_pool(name="stats", bufs=1))
    ps_pool = ctx.enter_context(tc.tile_pool(name="psum", bufs=2, space="PSUM"))
    ps3_pool = ctx.enter_context(tc.tile_pool(name="psum3", bufs=2, space="PSUM"))
    trp_pool = ctx.enter_context(tc.tile_pool(name="psumtr", bufs=2,
                                              space="PSUM"))

    wt1 = wpool.tile([P, 7, P], FP16, name="wt1")
    wt2 = wpool.tile([P, O2], FP16, name="wt2")
    wt3 = wpool.tile([P, 16], FP16, name="wt3")
    ident = wpool.tile([P, P], FP16, name="ident")
    corr2 = wpool.tile([1, O2], FP16, name="corr2")
    corr3 = wpool.tile([1, 16], FP16, name="corr3")
    ones1536 = wpool.tile([1, P], FP16, name="ones1536")
    nc.sync.dma_start(wt1[:], aps["wt1"].rearrange("(b p) o -> p b o", p=P))
    nc.sync.dma_start(wt2[:], aps["wt2"][:, :])
    nc.sync.dma_start(wt3[:], aps["wt3"][:, :])
    nc.sync.dma_start(ident[:], aps["ident"][:, :])
    nc.sync.dma_start(corr2[:], aps["corr2"][:, :])
    nc.sync.dma_start(corr3[:], aps["corr3"][:, :])
    nc.gpsimd.memset(ones1536[:], OFF)
    isw127 = {l: float(np.float32(isw[l]) / np.float32(127.0)) for l in isw}

    if general:
        # replicate per-feature scales across all partitions (DMA broadcast)
        sc1 = wpool.tile([P, D1], F32, name="sc1")
        sc2 = wpool.tile([P, D2], F32, name="sc2")
        sc3 = wpool.tile([P, D3], F32, name="sc3")
        for t_, ap_ in ((sc1, aps["scale1"]), (sc2, aps["scale2"]),
                        (sc3, aps["scale3"])):
            nc.sync.dma_start(t_[:], ap_[None, :].to_broadcast((P, ap_.shape[0])))

    st = {}
    for nm in ("mx1", "ss1", "c1", "b1", "mx2", "ss2", "c2", "b2",
               "mx3", "ss3", "c3", "b3",
               "tmpa1", "tmpb1", "tmpa2", "tmpb2", "tmpc2",
               "tmpa3", "tmpb3", "tmpc3"):
        st[nm] = stat_pool.tile([P, NT], F32, name=f"st_{nm}")
    outsb = stat_pool.tile([P, NT, O3], F32, name="outsb")
    sq_dump = stat_pool.tile([P, D1], F32, name="sq_dump")

    x_v = aps["x"].rearrange("(p t) c -> p t c", p=P)
    out_v = aps["out"].rearrange("(p t) o -> p t o", p=P)

    xb_pool = ctx.enter_context(tc.tile_pool(name="xblk", bufs=1))
    hc_pool = ctx.enter_context(tc.tile_pool(name="hchunk", bufs=1))
    q_pool = ctx.enter_context(tc.tile_pool(name="q", bufs=1))
    q23_pool = ctx.enter_context(tc.tile_pool(name="q23", bufs=1))
    qt_pool = ctx.enter_context(tc.tile_pool(name="qt", bufs=2))

    # x slots hold only the 784 real columns (f32)
    NBLK_X = (2 * (SB // TB) + 2) if not general else (SB // TB + 1)
    if NBLK_X_OVR is not None and not general:
        NBLK_X = NBLK_X_OVR
    x_slots = [xb_pool.tile([P, TB, D1], F32, name=f"xslot{i}")
               for i in range(NBLK_X)]

    # q1 slots: quant writes cols :784; col 784 is the constant +1536
    # correction input, cols 785+ stay 0 (for the xbar-transpose path whose
    # K-chunk 6 is a full 128 rows)
    NBLK_Q = 4
    q1_slots = [q_pool.tile([P, TB, K1], FP16, name=f"q1slot{i}")
                for i in range(NBLK_Q)]
    for qs in q1_slots:
        nc.gpsimd.memset(qs[:, :, D1 + 1:], 0.0)
        nc.gpsimd.memset(qs[:, :, D1:D1 + 1], OFF)

    h1_slots = [hc_pool.tile([P, SB, P], FP16, name=f"h1slot{i}")
                for i in range(HSLOTS)]
    BF16 = mybir.dt.bfloat16
    sq2_slots = [hc_pool.tile([P, SB, P], BF16, name=f"sq2slot{i}")
                 for i in range(HSLOTS)]
    sq3_slots = [hc_pool.tile([P, SB, D3], BF16, name=f"sq3slot{i}")
                 for i in range(HSLOTS)]
    h2_slots = [hc_pool.tile([P, SB, P], FP16, name=f"h2slot{i}")
                for i in range(HSLOTS)]
    for hs in h2_slots:
        nc.gpsimd.memset(hs[:, :, D3:], 0.0)   # pad cols stay 0 forever
    if general:
        xs_sc = [xb_pool.tile([P, TB, D1], F32, name=f"xscslot{i}")
                 for i in range(NBLK_X)]
        hsc_slots = [hc_pool.tile([P, SB, P], FP16, name=f"hsc{i}")
                     for i in range(HSLOTS)]
        for t_ in hsc_slots:
            nc.vector.memset(t_[:, :, :], 0.0)

    def stats_c(l, s0, s1):
        # critical-path quant-grid scale: c_l = 127 / max(mx_l, TINY)
        sl = (slice(None), slice(s0, s1))
        tmpb = st[f"tmpb{l}"][sl]
        mx = st[f"mx{l}"][sl]
        nc.vector.tensor_scalar_max(tmpb, mx, TINY)
        nc.vector.reciprocal(tmpb, tmpb)
        nc.vector.tensor_scalar_mul(st[f"c{l}"][sl], tmpb, 127.0)

    def stats_b(l, s0, s1):
        # deferred output-scale chain: b_l = max(ratio, EPS_Q) * isw/127
        sl = (slice(None), slice(s0, s1))
        tmpa = st[f"tmpa{l}"][sl]
        mx, ss = st[f"mx{l}"][sl], st[f"ss{l}"][sl]
        nc.scalar.activation(tmpa, ss, AF.Sqrt)
        if l == 1:
            nc.vector.tensor_scalar(tmpa, tmpa, RSQ_D[1], EPS_RMS,
                                    op0=OP.mult, op1=OP.add)   # D = rms+eps
            nc.vector.reciprocal(tmpa, tmpa)
            nc.vector.tensor_tensor(tmpa, mx, tmpa, op=OP.mult)  # ratio
        else:
            gp = st[f"b{l - 1}"][sl]
            tmpc = st[f"tmpc{l}"][sl]
            nc.vector.tensor_scalar_mul(tmpa, tmpa, RSQ_D[l])
            nc.vector.tensor_tensor(tmpa, tmpa, gp, op=OP.mult)  # true rms
            nc.vector.tensor_scalar_add(tmpa, tmpa, EPS_RMS)
            nc.vector.reciprocal(tmpa, tmpa)
            nc.vector.tensor_tensor(tmpc, mx, gp, op=OP.mult)    # true max
            nc.vector.tensor_tensor(tmpa, tmpc, tmpa, op=OP.mult)  # ratio
        nc.vector.tensor_scalar_max(tmpa, tmpa, EPS_Q)
        nc.vector.tensor_scalar_mul(st[f"b{l}"][sl], tmpa, isw127[l])

    NCH = NT // SB
    G2 = min(SB, 8)              # L2 psum-batch group size (N=G2*64 <= 512)

    def stage_load(ch):
        c0 = ch * SB
        for blk in range(c0 // TB, (c0 + SB) // TB):
            t0 = blk * TB
            xs = x_slots[blk % NBLK_X]
            nc.sync.dma_start(xs[:], x_v[:, t0:t0 + TB, :])

    def stage_astat(ch):
        c0 = ch * SB
        for blk in range(c0 // TB, (c0 + SB) // TB):
            t0 = blk * TB
            xs = x_slots[blk % NBLK_X]
            if general:
                xc = xs_sc[blk % NBLK_X]
                for i in range(TB):
                    nc.vector.tensor_tensor(xc[:, i, :], xs[:, i, :],
                                            sc1[:, :], op=OP.mult)
                qsrc = xc
            else:
                qsrc = xs
            nc.vector.tensor_reduce(st["mx1"][:, t0:t0 + TB], qsrc[:, :, :],
                                    axis=AX.X, op=OP.max,
                                    apply_absolute_value=True)
            if general:   # shallow slot depth: ss1 must run before slot reuse
                for i in range(TB):
                    nc.scalar.activation(sq_dump[:], xs[:, i, :], AF.Square,
                                         accum_out=st["ss1"][:, t0 + i:t0 + i + 1])
        stats_c(1, c0, c0 + SB)

    def _l1_block_dma(q1s, ps1):
        # DMA xbar transposes the whole TB-tile block in one instruction
        qt1 = qt_pool.tile([P, TB * 7, P], FP16, name="qt1t", tag="qt1")
        nc.sync.dma_start_transpose(qt1[:], q1s[:])
        for i in range(TB):
            for b in range(7):
                nc.tensor.matmul(ps1[:, i, :], lhsT=qt1[:, i * 7 + b, :],
                                 rhs=wt1[:, b, :],
                                 start=(b == 0), stop=(b == 6))

    def _l1_tile_pe_tr(q1s, i):
        # PE transpose of one 128-token tile into a PSUM bank (fp16)
        trp = trp_pool.tile([P, 7, P], FP16, name="trp", tag="trp")
        for b in range(6):
            nc.tensor.transpose(trp[:, b, :], q1s[:, i, b * P:(b + 1) * P],
                                ident[:])
        nc.tensor.transpose(trp[0:17, 6, :], q1s[:, i, 6 * P:6 * P + 17],
                            ident[:])
        return trp

    def _l1_tile_evac(trp, t):
        qt = qt_pool.tile([P, 7, P], FP16, name="qt1p", tag="qt1p")
        r = t % EV_ACT_MOD
        if r == 0:
            eng = nc.scalar.copy
        elif r == 1 and EV_GP:
            eng = nc.gpsimd.tensor_copy
        else:
            eng = nc.vector.tensor_copy
        eng(qt[:, 0:6, :], trp[:, 0:6, :])
        eng(qt[0:17, 6, :], trp[0:17, 6, :])
        return qt

    def _l1_tile_mm(qt, ps1, i):
        for b in range(6):
            nc.tensor.matmul(ps1[:, i, :], lhsT=qt[:, b, :], rhs=wt1[:, b, :],
                             start=(b == 0), stop=False)
        nc.tensor.matmul(ps1[:, i, :], lhsT=qt[0:17, 6, :],
                         rhs=wt1[0:17, 6, :], start=False, stop=True)

    def stage_quant(ch):
        c0 = ch * SB
        for jb in range(SB // TB):
            b0 = c0 + jb * TB
            blk = b0 // TB
            qsrc = (xs_sc if general else x_slots)[blk % NBLK_X]
            q1s = q1_slots[blk % NBLK_Q]
            for i in range(TB):
                csl = st["c1"][:, b0 + i:b0 + i + 1]
                if Q1_ACT_MOD and ((b0 + i) % Q1_ACT_MOD) == 0:
                    nc.scalar.activation(q1s[:, i, :D1], qsrc[:, i, :],
                                         AF.Copy, bias=OFF, scale=csl)
                else:
                    nc.gpsimd.tensor_scalar(q1s[:, i, :D1], qsrc[:, i, :],
                                            csl, OFF, op0=OP.mult, op1=OP.add)

    def stage_trmm(ch):
        c0 = ch * SB
        h1c = h1_slots[ch % HSLOTS]
        sq2c = sq2_slots[ch % HSLOTS]
        for jb in range(SB // TB):
            b0 = c0 + jb * TB
            blk = b0 // TB
            q1s = q1_slots[blk % NBLK_Q]
            ps1 = ps_pool.tile([P, TB, O1], F32, name="ps1")
            use_pe = ((blk % PE_TR_PERIOD) == PE_TR_PHASE
                      if PE_TR_HEAD is None else blk < PE_TR_HEAD)
            if use_pe:
                # PE-transpose path, software-pipelined tr/evac/mm per tile
                pend = []
                for i in range(TB):
                    trp = _l1_tile_pe_tr(q1s, i)
                    qt = _l1_tile_evac(trp, b0 + i)
                    pend.append((qt, i))
                    if len(pend) == 2:
                        qt0, i0 = pend.pop(0)
                        _l1_tile_mm(qt0, ps1, i0)
                for qt0, i0 in pend:
                    _l1_tile_mm(qt0, ps1, i0)
            else:
                _l1_block_dma(q1s, ps1)
            nc.scalar.activation(h1c[:, jb * TB:(jb + 1) * TB, :], ps1[:],
                                 AF.Relu)
            if general:
                hsc = hsc_slots[ch % HSLOTS]
                for i in range(TB):
                    j = jb * TB + i
                    nc.vector.tensor_tensor(hsc[:, j, :], h1c[:, j, :],
                                            sc2[:, :], op=OP.mult)
        # ---- deferred bulk ss1 (x slots still alive; after the relus so
        # the critical ACT ops win the queue order) ----
        for blk in ([] if general else range(c0 // TB, (c0 + SB) // TB)):
            t0 = blk * TB
            xs = x_slots[blk % NBLK_X]
            for i in range(TB):
                nc.scalar.activation(sq_dump[:], xs[:, i, :], AF.Square,
                                     accum_out=st["ss1"][:, t0 + i:t0 + i + 1])
        # ---- L2 row stats: critical mx2/c2 first, bulk ss2 after ----
        src2 = hsc_slots[ch % HSLOTS] if general else h1c
        nc.vector.tensor_reduce(st["mx2"][:, c0:c0 + SB], src2[:],
                                axis=AX.X, op=OP.max,
                                apply_absolute_value=general)
        stats_c(2, c0, c0 + SB)
        nc.vector.tensor_tensor(sq2c[:], h1c[:], h1c[:], op=OP.mult)
        nc.vector.tensor_reduce(st["ss2"][:, c0:c0 + SB], sq2c[:],
                                axis=AX.X, op=OP.add)

    q2_slots = [q23_pool.tile([P, SB, P], FP16, name=f"q2s{i}")
                for i in range(2)]
    q3_slots = [q23_pool.tile([P, SB, P], FP16, name=f"q3s{i}")
                for i in range(2)]

    def stage_q2(ch):
        c0 = ch * SB
        h1c = h1_slots[ch % HSLOTS]
        src2 = hsc_slots[ch % HSLOTS] if general else h1c
        q2 = q2_slots[ch % 2]
        qeng = nc.gpsimd if Q23_ENGINE == "gpsimd" else nc.vector
        if Q23_TAIL_VEC is not None and ch >= Q23_TAIL_VEC:
            qeng = nc.vector
        for j in range(SB):
            t = c0 + j
            qeng.tensor_scalar(q2[:, j, :], src2[:, j, :],
                                    st["c2"][:, t:t + 1],
                                    OFF, op0=OP.mult, op1=OP.add)

    def stage_l2(ch):
        c0 = ch * SB
        h2c = h2_slots[ch % HSLOTS]
        sq3c = sq3_slots[ch % HSLOTS]
        q2 = q2_slots[ch % 2]
        qt2 = qt_pool.tile([P, SB, P], FP16, name="qt2t", tag="qt2")
        H = SB // 2
        nc.sync.dma_start_transpose(qt2[:, :H, :], q2[:, :H, :])
        nc.sync.dma_start_transpose(qt2[:, H:, :], q2[:, H:, :])
        for g in range(SB // G2):
            ps2 = ps_pool.tile([P, G2, O2], F32, name="ps2")
            for jj in range(G2):
                j = g * G2 + jj
                nc.tensor.matmul(ps2[:, jj, :], lhsT=qt2[:, j, :], rhs=wt2[:],
                                 start=True, stop=False)
                nc.tensor.matmul(ps2[:, jj, :], lhsT=ones1536[:],
                                 rhs=corr2[:], start=False, stop=True)
            nc.scalar.activation(h2c[:, g * G2:(g + 1) * G2, :D3], ps2[:],
                                 AF.Relu)
        # ---- L3 row stats (chunk-batched) ----
        if general:
            hsc = hsc_slots[ch % HSLOTS]
            nc.vector.memset(hsc[:, :, D3:], 0.0)   # pads -> quant to 1536-OFF
            for j in range(SB):
                nc.vector.tensor_tensor(hsc[:, j, :D3], h2c[:, j, :D3],
                                        sc3[:, :], op=OP.mult)
            src3 = hsc
        else:
            src3 = h2c
        nc.vector.tensor_reduce(st["mx3"][:, c0:c0 + SB], src3[:, :, :D3],
                                axis=AX.X, op=OP.max,
                                apply_absolute_value=general)
        stats_c(3, c0, c0 + SB)
        nc.vector.tensor_tensor(sq3c[:], h2c[:, :, :D3], h2c[:, :, :D3],
                                op=OP.mult)
        nc.vector.tensor_reduce(st["ss3"][:, c0:c0 + SB], sq3c[:],
                                axis=AX.X, op=OP.add)

    def stage_q3(ch):
        c0 = ch * SB
        h2c = h2_slots[ch % HSLOTS]
        src3 = hsc_slots[ch % HSLOTS] if general else h2c
        q3 = q3_slots[ch % 2]
        qeng = nc.gpsimd if Q23_ENGINE == "gpsimd" else nc.vector
        if Q23_TAIL_VEC is not None and ch >= Q23_TAIL_VEC:
            qeng = nc.vector
        for j in range(SB):
            t = c0 + j
            qeng.tensor_scalar(q3[:, j, :], src3[:, j, :],
                                    st["c3"][:, t:t + 1],
                                    OFF, op0=OP.mult, op1=OP.add)

    def stage_bstat(ch):
        c0 = ch * SB
        stats_b(1, c0, c0 + SB)
        stats_b(2, c0, c0 + SB)
        stats_b(3, c0, c0 + SB)

    def stage_l3(ch):
        c0 = ch * SB
        q3 = q3_slots[ch % 2]
        qt3 = qt_pool.tile([P, SB, P], FP16, name="qt3t", tag="qt3")
        H = SB // 2
        nc.sync.dma_start_transpose(qt3[:, :H, :], q3[:, :H, :])
        nc.sync.dma_start_transpose(qt3[:, H:, :], q3[:, H:, :])
        ps3 = ps3_pool.tile([P, SB, 16], F32, name="ps3")
        for j in range(SB):
            nc.tensor.matmul(ps3[:, j, :], lhsT=qt3[:, j, :], rhs=wt3[:],
                             start=True, stop=False)
            nc.tensor.matmul(ps3[:, j, :], lhsT=ones1536[:],
                             rhs=corr3[:], start=False, stop=True)
        # final scale: out = z3' * b3 (per-row broadcast along o)
        oeng = nc.gpsimd if OUT_ENG == "gpsimd" else nc.vector
        oeng.tensor_tensor(
            outsb[:, c0:c0 + SB, :], ps3[:, :, :O3],
            st["b3"][:, c0:c0 + SB, None].to_broadcast((P, SB, O3)),
            op=OP.mult)

    # Software-pipelined emission: skew the four stages across chunks so
    # every engine's in-order instruction stream always has ready work.
    fns = (stage_load, stage_astat, stage_quant, stage_trmm,
           stage_q2, stage_l2, stage_q3, stage_bstat, stage_l3)
    stages = tuple(zip(fns, SKEW))
    for step in range(repeat * NCH + SKEW[-1]):
        for fn, off in stages:
            k = step - off
            if 0 <= k < repeat * NCH:
                fn(k % NCH)

    nc.sync.dma_start(out_v[:, :, :], outsb[:, :, :])


def _build_nc(R, isw, general_scales, TB=4, SB=8, repeat=1):
    nc = bacc.Bacc("TRN2", target_bir_lowering=False, debug=False)
    aps = {
        "x": nc.dram_tensor("x", [R, D1], F32, kind="ExternalInput").ap(),
        "wt1": nc.dram_tensor("wt1", [K1, O1], FP16, kind="ExternalInput").ap(),
        "wt2": nc.dram_tensor("wt2", [P, O2], FP16, kind="ExternalInput").ap(),
        "wt3": nc.dram_tensor("wt3", [P, 16], FP16, kind="ExternalInput").ap(),
        "ident": nc.dram_tensor("ident", [P, P], FP16,
                                kind="ExternalInput").ap(),
        "corr2": nc.dram_tensor("corr2", [1, O2], FP16,
                                kind="ExternalInput").ap(),
        "corr3": nc.dram_tensor("corr3", [1, 16], FP16,
                                kind="ExternalInput").ap(),
        "out": nc.dram_tensor("out", [R, O3], F32, kind="ExternalOutput").ap(),
    }
    if general_scales:
        aps["scale1"] = nc.dram_tensor("scale1", [D1], F32,
                                       kind="ExternalInput").ap()
        aps["scale2"] = nc.dram_tensor("scale2", [D2], F32,
                                       kind="ExternalInput").ap()
        aps["scale3"] = nc.dram_tensor("scale3", [D3], F32,
                                       kind="ExternalInput").ap()
    with tile.TileContext(nc) as tc:
        with ExitStack() as ctx:
            _ffn_body(ctx, tc, aps, R, isw,
                      scales=general_scales, TB=TB, SB=SB, repeat=repeat)
    nc.finalize()
    return nc


def kernel(x, w1, scale1, w2, scale2, w3, scale3, **_unused):
    x = np.ascontiguousarray(np.asarray(x, dtype=np.float32))
    w1 = np.asarray(w1, dtype=np.float32)
    w2 = np.asarray(w2, dtype=np.float32)
    w3 = np.asarray(w3, dtype=np.float32)
    scale1 = np.asarray(scale1, dtype=np.float32)
    scale2 = np.asarray(scale2, dtype=np.float32)
    scale3 = np.asarray(scale3, dtype=np.float32)

    B = x.shape[0]
    assert B % N_CORES == 0
    R = B // N_CORES

    arrays, isw = _host_weight_tensors(w1, w2, w3)
    ones = (np.all(scale1 == 1.0) and np.all(scale2 == 1.0)
            and np.all(scale3 == 1.0))
    general = None if ones else True

    nc = _build_nc(R, isw, general_scales=general)

    in_maps = []
    for i in range(N_CORES):
        m = {"x": x[i * R:(i + 1) * R], **arrays}
        if general:
            m["scale1"] = scale1
            m["scale2"] = scale2
            m["scale3"] = scale3
        in_maps.append(m)

    trace = bool(os.environ.get("FFN_TRACE"))
    res = run_bass_kernel_spmd(nc, in_maps, list(range(N_CORES)),
                               trace=trace,
                               tmpdir=os.environ.get("FFN_TRACE_DIR"))
    global LAST_EXEC_NS, LAST_TRACE
    LAST_EXEC_NS = res.exec_time_ns
    LAST_TRACE = res.instructions_and_trace
    out = np.concatenate([res.results[i]["out"] for i in range(N_CORES)],
                         axis=0)
    return out.astype(np.float32)


LAST_EXEC_NS = None
LAST_TRACE = None
